# revision 8
# baseline (speedup 1.0000x reference)
"""L1-distance attention kernel for Trainium2 (8 NeuronCores, SPMD).

Problem: q, k: [B=2, T=512, H=8, D=64] fp32
         out[b,s,t,h] = -sum_d |q[b,s,h,d] - k[b,t,h,d]| / sqrt(D)

Sharding: 16 (b,h) pairs across 8 cores, 2 pairs per core, stacked in the
SBUF partition dim (pair0 -> partitions 0:64 holding d, pair1 -> 64:128).

Per core, per query s (512 total), a producer computes a [128, T] tile:
  - ScalarE slots: activation Abs(bias=q[:,s], scale=-1) = |q_s - k|
    (exact; matmul weight -scale, no corrections), or
  - VectorE slots: tensor_scalar_min(k, q[:,s]) = min(k, q_s)
    (matmul weight +2*scale; needs -scale*K_t and -scale*Q_s corrections
    since |q-k| = q + k - 2*min(q,k)).
Then one float32r matmul per query contracts over d with a "strip"
stationary [128, 128] (a sliding window into a [128, 256] zero tile
with the weight at columns 128/129 on the pair0/pair1 partition
halves), routing query i's pair sums to PSUM partitions 2i / 2i+1.
float32r runs the PE at 1 cycle/row (vs 4 for fp32) but requires the
PSUM destination to start at partition 0 - hence the full-width strip
stationary instead of 32-row column tiling.

Per group of 64 queries: one wk matmul (moving = k) adds -scale*K_t to
the DVE-produced rows, then 64 query matmuls accumulate into one
[128, T] PSUM bank. The finished bank is copied to SBUF with the
per-row -scale*Q_s bias folded in (ScalarE Identity+bias / VectorE
tensor_scalar_add alternating) and DMAd out.

Host builds the strips/wk/qs and unscrambles output rows (row 2i+p of
group g = query 64g+i, pair p).
"""

import os

import numpy as np

os.environ.setdefault("MYCRO_LOCAL_CACHE", "1")

B, T, H, D = 2, 512, 8, 64
NCORES = 8
NGROUPS = 8  # query groups of 64 -> one PSUM bank each
SCALE = 1.0 / float(np.sqrt(np.float64(D)))  # 0.125

# Producer assignment per slot i in a group of 64: DVE tensor_scalar_min
# ~327ns (fp32 2x_2p), ScalarE activation Abs ~612ns, Pool (gpsimd)
# tensor_scalar_min ~711ns. With float32r matmuls PE needs ~111us
# (the bottleneck); keep every producer engine under that: per group
# 21 ACT / 3 POOL / 40 DVE -> ACT ~105us, DVE ~107us, POOL ~17us.
def _slot_engines():
    eng = []
    for i in range(64):
        eng.append("A" if (i * 21) % 64 < 21 else "D")
    non_act = [i for i in range(64) if eng[i] == "D"]
    for j in (non_act[7], non_act[21], non_act[35]):
        eng[j] = "P"
    return eng

SLOT_ENG = _slot_engines()


def slot_is_act(i):
    return SLOT_ENG[i] == "A"

_cached = {}


def _build_module(reps=1):
    from concourse import bacc, tile
    import concourse.mybir as mybir

    f32 = mybir.dt.float32
    f32r = mybir.dt.float32r
    nc = bacc.Bacc(
        "TRN2",
        target_bir_lowering=False,
        debug=False,
        enable_asserts=False,
        num_devices=1,
    )
    q_dram = nc.dram_tensor("q", [128, T], f32, kind="ExternalInput")
    k_dram = nc.dram_tensor("k", [128, T], f32r, kind="ExternalInput")
    ws_dram = nc.dram_tensor("ws", [128, 2, 256], f32r, kind="ExternalInput")
    wk_dram = nc.dram_tensor("wk", [128, 128], f32r, kind="ExternalInput")
    qs_dram = nc.dram_tensor("qs", [128, NGROUPS], f32, kind="ExternalInput")
    out_dram = nc.dram_tensor("out", [NGROUPS, 128, T], f32, kind="ExternalOutput")

    with tile.TileContext(nc) as tc:
        with (
            tc.tile_pool(name="const", bufs=1) as cpool,
            tc.tile_pool(name="ad", bufs=14) as adpool,
            tc.tile_pool(name="osb", bufs=3) as opool,
            tc.tile_pool(name="psum", bufs=8, space="PSUM") as ppool,
        ):
            q_sb = cpool.tile([128, T], f32, tag="q")
            k_sb = cpool.tile([128, T], f32r, tag="k")
            ws_sb = cpool.tile([128, 2, 256], f32r, tag="ws")
            wk_sb = cpool.tile([128, 128], f32r, tag="wk")
            qs_sb = cpool.tile([128, NGROUPS], f32, tag="qs")
            nc.sync.dma_start(k_sb[:], k_dram[:])
            nc.sync.dma_start(q_sb[:], q_dram[:])
            nc.sync.dma_start(wk_sb[:], wk_dram[:])
            nc.sync.dma_start(ws_sb[:], ws_dram[:])
            nc.sync.dma_start(qs_sb[:], qs_dram[:])

            for g in range(NGROUPS * reps):
                g = g % NGROUPS
                psum_t = ppool.tile([128, T], f32, tag="acc")
                # -scale*K_t for the DVE-produced rows (ACT columns are 0)
                nc.tensor.matmul(
                    psum_t[:],
                    wk_sb[:],
                    k_sb[:],
                    start=True,
                    stop=False,
                )
                for i in range(64):
                    s = 64 * g + i
                    ad = adpool.tile([128, T], f32r, tag="ad")
                    eng = SLOT_ENG[i]
                    if eng == "A":
                        nc.scalar.activation(
                            ad[:],
                            k_sb[:],
                            mybir.ActivationFunctionType.Abs,
                            bias=q_sb[:, s : s + 1],
                            scale=-1.0,
                        )
                        wsel = ws_sb[:, 0]
                    elif eng == "P":
                        nc.gpsimd.tensor_scalar_min(
                            ad[:], k_sb[:], q_sb[:, s : s + 1]
                        )
                        wsel = ws_sb[:, 1]
                    else:
                        nc.vector.tensor_scalar_min(
                            ad[:], k_sb[:], q_sb[:, s : s + 1]
                        )
                        wsel = ws_sb[:, 1]
                    nc.tensor.matmul(
                        psum_t[:],
                        wsel[:, 128 - 2 * i : 256 - 2 * i],
                        ad[:],
                        start=False,
                        stop=(i == 63),
                    )
                ob = opool.tile([128, T], f32, tag="ob")
                if g % 2 == 0:
                    # copy + per-row bias (-scale*Q_s on DVE rows) on ScalarE
                    nc.scalar.activation(
                        ob[:],
                        psum_t[:],
                        mybir.ActivationFunctionType.Identity,
                        bias=qs_sb[:, g : g + 1],
                        scale=1.0,
                    )
                else:
                    nc.vector.tensor_scalar_add(
                        ob[:], psum_t[:], qs_sb[:, g : g + 1]
                    )
                nc.sync.dma_start(out_dram[g], ob[:])

    nc.compile()
    return nc


def _host_weights():
    # ws[0]: ACT strip (-scale), ws[1]: DVE strip (+2*scale); nonzero at
    # columns 128 (pair0 partitions) and 129 (pair1 partitions).
    ws = np.zeros((128, 2, 256), np.float32)
    ws[0:64, 0, 128] = -SCALE
    ws[64:128, 0, 129] = -SCALE
    ws[0:64, 1, 128] = 2.0 * SCALE
    ws[64:128, 1, 129] = 2.0 * SCALE
    # wk: column r = 2i+p gets -scale on pair-p partitions for DVE slots.
    wk = np.zeros((128, 128), np.float32)
    for i in range(64):
        if not slot_is_act(i):
            wk[0:64, 2 * i] = -SCALE
            wk[64:128, 2 * i + 1] = -SCALE
    return ws, wk


def _host_qsum(qc):
    """qc: [128, T] per-core stacked q^T. Returns qs [128, NGROUPS] fp32:
    row r = 2i+p of group g gets -scale*sum_d q[pair p, d, s] for DVE
    slots (s = 64g+i), 0 for ACT slots."""
    qsum = qc.astype(np.float64).reshape(2, 64, T).sum(axis=1)  # [pair, s]
    qs = np.zeros((128, NGROUPS), np.float64)
    for g in range(NGROUPS):
        for i in range(64):
            if not slot_is_act(i):
                s = 64 * g + i
                for p in range(2):
                    qs[2 * i + p, g] = -SCALE * qsum[p, s]
    return qs.astype(np.float32)


def get_module(reps=1):
    key = ("nc", reps)
    nc = _cached.get(key)
    if nc is None:
        nc = _build_module(reps)
        _cached[key] = nc
    return nc


def make_in_maps(q, k):
    """Shard full [B,T,H,D] q/k into 8 per-core input maps."""
    q = np.asarray(q, dtype=np.float32)
    k = np.asarray(k, dtype=np.float32)
    # [B, T, H, D] -> [B, H, D, T] -> [B*H, D, T]
    qt = np.ascontiguousarray(q.transpose(0, 2, 3, 1)).reshape(B * H, D, T)
    kt = np.ascontiguousarray(k.transpose(0, 2, 3, 1)).reshape(B * H, D, T)
    ws, wk = _host_weights()
    in_maps = []
    for c in range(NCORES):
        qc = np.ascontiguousarray(qt[2 * c : 2 * c + 2].reshape(128, T))
        kc = np.ascontiguousarray(kt[2 * c : 2 * c + 2].reshape(128, T))
        in_maps.append(
            {"q": qc, "k": kc, "ws": ws, "wk": wk, "qs": _host_qsum(qc)}
        )
    return in_maps


def assemble_output(core_outs):
    """core_outs: list of 8 arrays [NGROUPS, 128, T] -> full [B, T, T, H]."""
    outf = np.empty((B, T, T, H), np.float32)
    for c in range(NCORES):
        o = np.asarray(core_outs[c]).reshape(NGROUPS, 64, 2, T)
        # row r = 2i+p in group g  ->  query s = 64g + i, pair p
        o = o.transpose(2, 0, 1, 3).reshape(2, T, T)
        for p in range(2):
            pg = 2 * c + p
            b, h = divmod(pg, H)
            outf[b, :, :, h] = o[p]
    return outf


def kernel(q, k):
    from concourse.bass_utils import run_bass_kernel_spmd

    nc = get_module()
    in_maps = make_in_maps(q, k)
    res = run_bass_kernel_spmd(
        nc, in_maps, core_ids=list(range(NCORES)), trace=False
    )
    _cached["last_results"] = res
    return assemble_output([r["out"] for r in res.results])


# revision 9
# speedup vs baseline: 5.8970x; 5.8970x over previous
"""L1-distance attention kernel for Trainium2 (8 NeuronCores, SPMD).

Problem: q, k: [B=2, T=512, H=8, D=64] fp32
         out[b,s,t,h] = -sum_d |q[b,s,h,d] - k[b,t,h,d]| / sqrt(D)

Sharding: 16 (b,h) pairs across 8 cores, 2 pairs per core, stacked in the
SBUF partition dim (pair0 -> partitions 0:64 holding d, pair1 -> 64:128).

Per core, per query s (512 total), a producer computes a [128, T] tile:
  - ScalarE slots: activation Abs(bias=q[:,s], scale=-1) = |q_s - k|
    (exact; matmul weight -scale, no corrections), or
  - VectorE slots: tensor_scalar_min(k, q[:,s]) = min(k, q_s)
    (matmul weight +2*scale; needs -scale*K_t and -scale*Q_s corrections
    since |q-k| = q + k - 2*min(q,k)).
Then one float32r matmul per query contracts over d with a "strip"
stationary [128, 128] (a sliding window into a [128, 256] zero tile
with the weight at columns 128/129 on the pair0/pair1 partition
halves), routing query i's pair sums to PSUM partitions 2i / 2i+1.
float32r runs the PE at 1 cycle/row (vs 4 for fp32) but requires the
PSUM destination to start at partition 0 - hence the full-width strip
stationary instead of 32-row column tiling.

Per group of 64 queries: one wk matmul (moving = k) adds -scale*K_t to
the DVE-produced rows, then 64 query matmuls accumulate into one
[128, T] PSUM bank. The finished bank is copied to SBUF with the
per-row -scale*Q_s bias folded in (ScalarE Identity+bias / VectorE
tensor_scalar_add alternating) and DMAd out.

Host builds the strips/wk/qs and unscrambles output rows (row 2i+p of
group g = query 64g+i, pair p).
"""

import os

import numpy as np

os.environ.setdefault("MYCRO_LOCAL_CACHE", "1")

B, T, H, D = 2, 512, 8, 64
NCORES = 8
NGROUPS = 8  # query groups of 64 -> one PSUM bank each
SCALE = 1.0 / float(np.sqrt(np.float64(D)))  # 0.125

# Producer assignment per slot i in a group of 64: DVE tensor_scalar_min
# ~327ns (fp32 2x_2p), ScalarE activation Abs ~612ns. (gpsimd was tried
# as a third producer and is far slower on HW than its cost model.)
# With float32r matmuls PE needs ~115us (the bottleneck); 23 ACT /
# 41 DVE per group keeps ACT ~112.6us, DVE ~107.3us + 8 out-copies.
def slot_is_act(i):
    return (i * 23) % 64 < 23

_cached = {}


def _build_module(reps=1):
    from concourse import bacc, tile
    import concourse.mybir as mybir

    f32 = mybir.dt.float32
    f32r = mybir.dt.float32r
    nc = bacc.Bacc(
        "TRN2",
        target_bir_lowering=False,
        debug=False,
        enable_asserts=False,
        num_devices=1,
    )
    q_dram = nc.dram_tensor("q", [128, T], f32, kind="ExternalInput")
    k_dram = nc.dram_tensor("k", [128, T], f32r, kind="ExternalInput")
    ws_dram = nc.dram_tensor("ws", [128, 2, 256], f32r, kind="ExternalInput")
    wk_dram = nc.dram_tensor("wk", [128, 128], f32r, kind="ExternalInput")
    qs_dram = nc.dram_tensor("qs", [128, NGROUPS], f32, kind="ExternalInput")
    out_dram = nc.dram_tensor("out", [NGROUPS, 128, T], f32, kind="ExternalOutput")

    with tile.TileContext(nc) as tc:
        with (
            tc.tile_pool(name="const", bufs=1) as cpool,
            tc.tile_pool(name="ad", bufs=14) as adpool,
            tc.tile_pool(name="osb", bufs=3) as opool,
            tc.tile_pool(name="psum", bufs=8, space="PSUM") as ppool,
        ):
            q_sb = cpool.tile([128, T], f32, tag="q")
            k_sb = cpool.tile([128, T], f32r, tag="k")
            ws_sb = cpool.tile([128, 2, 256], f32r, tag="ws")
            wk_sb = cpool.tile([128, 128], f32r, tag="wk")
            qs_sb = cpool.tile([128, NGROUPS], f32, tag="qs")
            nc.sync.dma_start(k_sb[:], k_dram[:])
            nc.sync.dma_start(q_sb[:], q_dram[:])
            nc.sync.dma_start(wk_sb[:], wk_dram[:])
            nc.sync.dma_start(ws_sb[:], ws_dram[:])
            nc.sync.dma_start(qs_sb[:], qs_dram[:])

            for g in range(NGROUPS * reps):
                g = g % NGROUPS
                psum_t = ppool.tile([128, T], f32, tag="acc")
                # -scale*K_t for the DVE-produced rows (ACT columns are 0)
                nc.tensor.matmul(
                    psum_t[:],
                    wk_sb[:],
                    k_sb[:],
                    start=True,
                    stop=False,
                )
                for i in range(64):
                    s = 64 * g + i
                    ad = adpool.tile([128, T], f32r, tag="ad")
                    if slot_is_act(i):
                        nc.scalar.activation(
                            ad[:],
                            k_sb[:],
                            mybir.ActivationFunctionType.Abs,
                            bias=q_sb[:, s : s + 1],
                            scale=-1.0,
                        )
                        wsel = ws_sb[:, 0]
                    else:
                        nc.vector.tensor_scalar_min(
                            ad[:], k_sb[:], q_sb[:, s : s + 1]
                        )
                        wsel = ws_sb[:, 1]
                    nc.tensor.matmul(
                        psum_t[:],
                        wsel[:, 128 - 2 * i : 256 - 2 * i],
                        ad[:],
                        start=False,
                        stop=(i == 63),
                    )
                ob = opool.tile([128, T], f32, tag="ob")
                # copy + per-row bias (-scale*Q_s on DVE rows); all on
                # VectorE so ScalarE stays under the PE roofline
                nc.vector.tensor_scalar_add(
                    ob[:], psum_t[:], qs_sb[:, g : g + 1]
                )
                nc.sync.dma_start(out_dram[g], ob[:])

    nc.compile()
    return nc


def _host_weights():
    # ws[0]: ACT strip (-scale), ws[1]: DVE strip (+2*scale); nonzero at
    # columns 128 (pair0 partitions) and 129 (pair1 partitions).
    ws = np.zeros((128, 2, 256), np.float32)
    ws[0:64, 0, 128] = -SCALE
    ws[64:128, 0, 129] = -SCALE
    ws[0:64, 1, 128] = 2.0 * SCALE
    ws[64:128, 1, 129] = 2.0 * SCALE
    # wk: column r = 2i+p gets -scale on pair-p partitions for DVE slots.
    wk = np.zeros((128, 128), np.float32)
    for i in range(64):
        if not slot_is_act(i):
            wk[0:64, 2 * i] = -SCALE
            wk[64:128, 2 * i + 1] = -SCALE
    return ws, wk


def _host_qsum(qc):
    """qc: [128, T] per-core stacked q^T. Returns qs [128, NGROUPS] fp32:
    row r = 2i+p of group g gets -scale*sum_d q[pair p, d, s] for DVE
    slots (s = 64g+i), 0 for ACT slots."""
    qsum = qc.astype(np.float64).reshape(2, 64, T).sum(axis=1)  # [pair, s]
    qs = np.zeros((128, NGROUPS), np.float64)
    for g in range(NGROUPS):
        for i in range(64):
            if not slot_is_act(i):
                s = 64 * g + i
                for p in range(2):
                    qs[2 * i + p, g] = -SCALE * qsum[p, s]
    return qs.astype(np.float32)


def get_module(reps=1):
    key = ("nc", reps)
    nc = _cached.get(key)
    if nc is None:
        nc = _build_module(reps)
        _cached[key] = nc
    return nc


def make_in_maps(q, k):
    """Shard full [B,T,H,D] q/k into 8 per-core input maps."""
    q = np.asarray(q, dtype=np.float32)
    k = np.asarray(k, dtype=np.float32)
    # [B, T, H, D] -> [B, H, D, T] -> [B*H, D, T]
    qt = np.ascontiguousarray(q.transpose(0, 2, 3, 1)).reshape(B * H, D, T)
    kt = np.ascontiguousarray(k.transpose(0, 2, 3, 1)).reshape(B * H, D, T)
    ws, wk = _host_weights()
    in_maps = []
    for c in range(NCORES):
        qc = np.ascontiguousarray(qt[2 * c : 2 * c + 2].reshape(128, T))
        kc = np.ascontiguousarray(kt[2 * c : 2 * c + 2].reshape(128, T))
        in_maps.append(
            {"q": qc, "k": kc, "ws": ws, "wk": wk, "qs": _host_qsum(qc)}
        )
    return in_maps


def assemble_output(core_outs):
    """core_outs: list of 8 arrays [NGROUPS, 128, T] -> full [B, T, T, H]."""
    outf = np.empty((B, T, T, H), np.float32)
    for c in range(NCORES):
        o = np.asarray(core_outs[c]).reshape(NGROUPS, 64, 2, T)
        # row r = 2i+p in group g  ->  query s = 64g + i, pair p
        o = o.transpose(2, 0, 1, 3).reshape(2, T, T)
        for p in range(2):
            pg = 2 * c + p
            b, h = divmod(pg, H)
            outf[b, :, :, h] = o[p]
    return outf


def kernel(q, k):
    from concourse.bass_utils import run_bass_kernel_spmd

    nc = get_module()
    in_maps = make_in_maps(q, k)
    res = run_bass_kernel_spmd(
        nc, in_maps, core_ids=list(range(NCORES)), trace=False
    )
    _cached["last_results"] = res
    return assemble_output([r["out"] for r in res.results])


# revision 10
# speedup vs baseline: 10.3625x; 1.7572x over previous
"""L1-distance attention kernel for Trainium2 (8 NeuronCores, SPMD).

Problem: q, k: [B=2, T=512, H=8, D=64] fp32
         out[b,s,t,h] = -sum_d |q[b,s,h,d] - k[b,t,h,d]| / sqrt(D)

Sharding: 16 (b,h) pairs across 8 cores, 2 pairs per core, stacked in the
SBUF partition dim (pair0 -> partitions 0:64 holding d, pair1 -> 64:128).

Per core, per query s (512 total), a producer computes a [128, T]
bf16 tile (k is pre-rounded to bf16 on host; bf16 producers get the
DVE 4x mode and bf16 matmuls run the PE at 1 cycle/row):
  - ScalarE slots: activation Abs(bias=q[:,s], scale=-1) = |q_s - k|
    (matmul weight -scale, no corrections), or
  - VectorE slots: tensor_scalar_min(k, q[:,s]) = min(k, q_s)
    (matmul weight +2*scale; needs -scale*K_t and -scale*Q_s corrections
    since |q-k| = q + k - 2*min(q,k); K_t/Q_s use bf16-rounded values,
    which the min emits exactly).
Then one bf16 matmul per query contracts over d with a "strip"
stationary [128, 128] (a sliding window into a [128, 256] zero tile
with the weight at columns 128/129 on the pair0/pair1 partition
halves), routing query i's pair sums to PSUM partitions 2i / 2i+1.

Per group of 64 queries: one wk matmul (moving = k) adds -scale*K_t to
the DVE-produced rows, then 64 query matmuls accumulate into one
[128, T] PSUM bank. The finished bank is copied to SBUF with the
per-row -scale*Q_s bias folded in (ScalarE Identity+bias / VectorE
tensor_scalar_add alternating) and DMAd out.

Host builds the strips/wk/qs and unscrambles output rows (row 2i+p of
group g = query 64g+i, pair p).
"""

import os

import numpy as np

os.environ.setdefault("MYCRO_LOCAL_CACHE", "1")

B, T, H, D = 2, 512, 8, 64
NCORES = 8
NGROUPS = 8  # query groups of 64 -> one PSUM bank each
SCALE = 1.0 / float(np.sqrt(np.float64(D)))  # 0.125

# Producer assignment per slot i in a group of 64: DVE tensor_scalar_min
# ~194ns in bf16 (4x mode), ScalarE activation Abs ~612ns. (gpsimd was
# tried as a third producer and is far slower on HW than its cost
# model.) bf16 matmuls keep PE at ~111us (the bottleneck); 18 ACT /
# 46 DVE per group keeps ACT ~88us, DVE ~77us + 8 out-copies.
def slot_is_act(i):
    return (i * 18) % 64 < 18

_cached = {}


def _build_module(reps=1):
    from concourse import bacc, tile
    import concourse.mybir as mybir

    f32 = mybir.dt.float32
    bf16 = mybir.dt.bfloat16
    nc = bacc.Bacc(
        "TRN2",
        target_bir_lowering=False,
        debug=False,
        enable_asserts=False,
        num_devices=1,
    )
    q_dram = nc.dram_tensor("q", [128, T], f32, kind="ExternalInput")
    k_dram = nc.dram_tensor("k", [128, T], bf16, kind="ExternalInput")
    ws_dram = nc.dram_tensor("ws", [128, 2, 256], bf16, kind="ExternalInput")
    wk_dram = nc.dram_tensor("wk", [128, 128], bf16, kind="ExternalInput")
    qs_dram = nc.dram_tensor("qs", [128, NGROUPS], f32, kind="ExternalInput")
    out_dram = nc.dram_tensor("out", [NGROUPS, 128, T], f32, kind="ExternalOutput")

    with tile.TileContext(nc) as tc:
        with (
            tc.tile_pool(name="const", bufs=1) as cpool,
            tc.tile_pool(name="ad", bufs=14) as adpool,
            tc.tile_pool(name="osb", bufs=3) as opool,
            tc.tile_pool(name="psum", bufs=8, space="PSUM") as ppool,
        ):
            q_sb = cpool.tile([128, T], f32, tag="q")
            k_sb = cpool.tile([128, T], bf16, tag="k")
            ws_sb = cpool.tile([128, 2, 256], bf16, tag="ws")
            wk_sb = cpool.tile([128, 128], bf16, tag="wk")
            qs_sb = cpool.tile([128, NGROUPS], f32, tag="qs")
            nc.sync.dma_start(k_sb[:], k_dram[:])
            nc.sync.dma_start(q_sb[:], q_dram[:])
            nc.sync.dma_start(wk_sb[:], wk_dram[:])
            nc.sync.dma_start(ws_sb[:], ws_dram[:])
            nc.sync.dma_start(qs_sb[:], qs_dram[:])

            for g in range(NGROUPS * reps):
                g = g % NGROUPS
                psum_t = ppool.tile([128, T], f32, tag="acc")
                # -scale*K_t for the DVE-produced rows (ACT columns are 0)
                nc.tensor.matmul(
                    psum_t[:],
                    wk_sb[:],
                    k_sb[:],
                    start=True,
                    stop=False,
                )
                for i in range(64):
                    s = 64 * g + i
                    ad = adpool.tile([128, T], bf16, tag="ad")
                    if slot_is_act(i):
                        nc.scalar.activation(
                            ad[:],
                            k_sb[:],
                            mybir.ActivationFunctionType.Abs,
                            bias=q_sb[:, s : s + 1],
                            scale=-1.0,
                        )
                        wsel = ws_sb[:, 0]
                    else:
                        nc.vector.tensor_scalar_min(
                            ad[:], k_sb[:], q_sb[:, s : s + 1]
                        )
                        wsel = ws_sb[:, 1]
                    nc.tensor.matmul(
                        psum_t[:],
                        wsel[:, 128 - 2 * i : 256 - 2 * i],
                        ad[:],
                        start=False,
                        stop=(i == 63),
                    )
                ob = opool.tile([128, T], f32, tag="ob")
                # copy + per-row bias (-scale*Q_s on DVE rows); all on
                # VectorE so ScalarE stays under the PE roofline
                nc.vector.tensor_scalar_add(
                    ob[:], psum_t[:], qs_sb[:, g : g + 1]
                )
                nc.sync.dma_start(out_dram[g], ob[:])

    nc.compile()
    return nc


def _host_weights():
    import ml_dtypes

    # ws[0]: ACT strip (-scale), ws[1]: DVE strip (+2*scale); nonzero at
    # columns 128 (pair0 partitions) and 129 (pair1 partitions).
    ws = np.zeros((128, 2, 256), np.float32)
    ws[0:64, 0, 128] = -SCALE
    ws[64:128, 0, 129] = -SCALE
    ws[0:64, 1, 128] = 2.0 * SCALE
    ws[64:128, 1, 129] = 2.0 * SCALE
    # wk: column r = 2i+p gets -scale on pair-p partitions for DVE slots.
    wk = np.zeros((128, 128), np.float32)
    for i in range(64):
        if not slot_is_act(i):
            wk[0:64, 2 * i] = -SCALE
            wk[64:128, 2 * i + 1] = -SCALE
    bf16 = ml_dtypes.bfloat16
    return ws.astype(bf16), wk.astype(bf16)


def _host_qsum(qc):
    """qc: [128, T] per-core stacked q^T. Returns qs [128, NGROUPS] fp32:
    row r = 2i+p of group g gets -scale*sum_d q[pair p, d, s] for DVE
    slots (s = 64g+i), 0 for ACT slots."""
    import ml_dtypes

    qb = qc.astype(ml_dtypes.bfloat16).astype(np.float64)
    qsum = qb.reshape(2, 64, T).sum(axis=1)  # [pair, s]
    qs = np.zeros((128, NGROUPS), np.float64)
    for g in range(NGROUPS):
        for i in range(64):
            if not slot_is_act(i):
                s = 64 * g + i
                for p in range(2):
                    qs[2 * i + p, g] = -SCALE * qsum[p, s]
    return qs.astype(np.float32)


def get_module(reps=1):
    key = ("nc", reps)
    nc = _cached.get(key)
    if nc is None:
        nc = _build_module(reps)
        _cached[key] = nc
    return nc


def make_in_maps(q, k):
    """Shard full [B,T,H,D] q/k into 8 per-core input maps."""
    q = np.asarray(q, dtype=np.float32)
    k = np.asarray(k, dtype=np.float32)
    # [B, T, H, D] -> [B, H, D, T] -> [B*H, D, T]
    qt = np.ascontiguousarray(q.transpose(0, 2, 3, 1)).reshape(B * H, D, T)
    kt = np.ascontiguousarray(k.transpose(0, 2, 3, 1)).reshape(B * H, D, T)
    ws, wk = _host_weights()
    in_maps = []
    import ml_dtypes

    for c in range(NCORES):
        qc = np.ascontiguousarray(qt[2 * c : 2 * c + 2].reshape(128, T))
        kc = np.ascontiguousarray(
            kt[2 * c : 2 * c + 2].reshape(128, T)
        ).astype(ml_dtypes.bfloat16)
        in_maps.append(
            {"q": qc, "k": kc, "ws": ws, "wk": wk, "qs": _host_qsum(qc)}
        )
    return in_maps


def assemble_output(core_outs):
    """core_outs: list of 8 arrays [NGROUPS, 128, T] -> full [B, T, T, H]."""
    outf = np.empty((B, T, T, H), np.float32)
    for c in range(NCORES):
        o = np.asarray(core_outs[c]).reshape(NGROUPS, 64, 2, T)
        # row r = 2i+p in group g  ->  query s = 64g + i, pair p
        o = o.transpose(2, 0, 1, 3).reshape(2, T, T)
        for p in range(2):
            pg = 2 * c + p
            b, h = divmod(pg, H)
            outf[b, :, :, h] = o[p]
    return outf


def kernel(q, k):
    from concourse.bass_utils import run_bass_kernel_spmd

    nc = get_module()
    in_maps = make_in_maps(q, k)
    res = run_bass_kernel_spmd(
        nc, in_maps, core_ids=list(range(NCORES)), trace=False
    )
    _cached["last_results"] = res
    return assemble_output([r["out"] for r in res.results])


# revision 11
# speedup vs baseline: 10.5161x; 1.0148x over previous
"""L1-distance attention kernel for Trainium2 (8 NeuronCores, SPMD).

Problem: q, k: [B=2, T=512, H=8, D=64] fp32
         out[b,s,t,h] = -sum_d |q[b,s,h,d] - k[b,t,h,d]| / sqrt(D)

Sharding: 16 (b,h) pairs across 8 cores, 2 pairs per core, stacked in the
SBUF partition dim (pair0 -> partitions 0:64 holding d, pair1 -> 64:128).

Per core, per query s (512 total), a producer computes a [128, T]
bf16 tile (k is pre-rounded to bf16 on host; bf16 producers get the
DVE 4x mode and bf16 matmuls run the PE at 1 cycle/row):
  - ScalarE slots: activation Abs(bias=q[:,s], scale=-1) = |q_s - k|
    (matmul weight -scale, no corrections), or
  - VectorE slots: tensor_scalar_min(k, q[:,s]) = min(k, q_s)
    (matmul weight +2*scale; needs -scale*K_t and -scale*Q_s corrections
    since |q-k| = q + k - 2*min(q,k); K_t/Q_s use bf16-rounded values,
    which the min emits exactly).
Then one bf16 matmul per query contracts over d with a "strip"
stationary [128, 128] (a sliding window into a [128, 256] zero tile
with the weight at columns 128/129 on the pair0/pair1 partition
halves), routing query i's pair sums to PSUM partitions 2i / 2i+1.

Per group of 64 queries: one wk matmul (moving = k) adds -scale*K_t to
the DVE-produced rows, then 64 query matmuls accumulate into one
[128, T] PSUM bank. The finished bank is copied to SBUF with the
per-row -scale*Q_s bias folded in (ScalarE Identity+bias / VectorE
tensor_scalar_add alternating) and DMAd out.

Host builds the strips/wk/qs and unscrambles output rows (row 2i+p of
group g = query 64g+i, pair p).
"""

import os

import numpy as np

os.environ.setdefault("MYCRO_LOCAL_CACHE", "1")

B, T, H, D = 2, 512, 8, 64
NCORES = 8
NGROUPS = 8  # query groups of 64 -> one PSUM bank each
SCALE = 1.0 / float(np.sqrt(np.float64(D)))  # 0.125

# Producer assignment per slot i in a group of 64: DVE tensor_scalar_min
# ~194ns in bf16 (4x mode), ScalarE activation Abs ~612ns. (gpsimd was
# tried as a third producer and is far slower on HW than its cost
# model.) bf16 matmuls keep PE at ~111us (the bottleneck); 18 ACT /
# 46 DVE per group keeps ACT ~88us, DVE ~77us + 8 out-copies.
def slot_is_act(i):
    return (i * 15) % 64 < 15

_cached = {}


def _build_module(reps=1):
    from concourse import bacc, tile
    import concourse.mybir as mybir

    f32 = mybir.dt.float32
    bf16 = mybir.dt.bfloat16
    nc = bacc.Bacc(
        "TRN2",
        target_bir_lowering=False,
        debug=False,
        enable_asserts=False,
        num_devices=1,
    )
    q_dram = nc.dram_tensor("q", [128, T], f32, kind="ExternalInput")
    k_dram = nc.dram_tensor("k", [128, T], bf16, kind="ExternalInput")
    ws_dram = nc.dram_tensor("ws", [128, 2, 256], bf16, kind="ExternalInput")
    wk_dram = nc.dram_tensor("wk", [128, 128], bf16, kind="ExternalInput")
    qs_dram = nc.dram_tensor("qs", [128, NGROUPS], f32, kind="ExternalInput")
    out_dram = nc.dram_tensor("out", [NGROUPS, 128, T], f32, kind="ExternalOutput")

    with tile.TileContext(nc) as tc:
        with (
            tc.tile_pool(name="const", bufs=1) as cpool,
            tc.tile_pool(name="ad", bufs=14) as adpool,
            tc.tile_pool(name="osb", bufs=3) as opool,
            tc.tile_pool(name="psum", bufs=8, space="PSUM") as ppool,
        ):
            q_sb = cpool.tile([128, T], f32, tag="q")
            k_sb = cpool.tile([128, T], bf16, tag="k")
            ws_sb = cpool.tile([128, 2, 256], bf16, tag="ws")
            wk_sb = cpool.tile([128, 128], bf16, tag="wk")
            qs_sb = cpool.tile([128, NGROUPS], f32, tag="qs")
            nc.sync.dma_start(k_sb[:], k_dram[:])
            nc.sync.dma_start(q_sb[:], q_dram[:])
            nc.sync.dma_start(wk_sb[:], wk_dram[:])
            nc.sync.dma_start(ws_sb[:], ws_dram[:])
            nc.sync.dma_start(qs_sb[:], qs_dram[:])

            for g in range(NGROUPS * reps):
                g = g % NGROUPS
                psum_t = ppool.tile([128, T], f32, tag="acc")
                # -scale*K_t for the DVE-produced rows (ACT columns are 0)
                nc.tensor.matmul(
                    psum_t[:],
                    wk_sb[:],
                    k_sb[:],
                    start=True,
                    stop=False,
                )
                for i in range(64):
                    s = 64 * g + i
                    ad = adpool.tile([128, T], bf16, tag="ad")
                    if slot_is_act(i):
                        nc.scalar.activation(
                            ad[:],
                            k_sb[:],
                            mybir.ActivationFunctionType.Abs,
                            bias=q_sb[:, s : s + 1],
                            scale=-1.0,
                        )
                        wsel = ws_sb[:, 0]
                    else:
                        nc.vector.tensor_scalar_min(
                            ad[:], k_sb[:], q_sb[:, s : s + 1]
                        )
                        wsel = ws_sb[:, 1]
                    nc.tensor.matmul(
                        psum_t[:],
                        wsel[:, 128 - 2 * i : 256 - 2 * i],
                        ad[:],
                        start=False,
                        stop=(i == 63),
                    )
                ob = opool.tile([128, T], f32, tag="ob")
                # copy + per-row bias (-scale*Q_s on DVE rows); all on
                # VectorE so ScalarE stays under the PE roofline
                nc.vector.tensor_scalar_add(
                    ob[:], psum_t[:], qs_sb[:, g : g + 1]
                )
                nc.sync.dma_start(out_dram[g], ob[:])

    nc.compile()
    return nc


def _host_weights():
    import ml_dtypes

    # ws[0]: ACT strip (-scale), ws[1]: DVE strip (+2*scale); nonzero at
    # columns 128 (pair0 partitions) and 129 (pair1 partitions).
    ws = np.zeros((128, 2, 256), np.float32)
    ws[0:64, 0, 128] = -SCALE
    ws[64:128, 0, 129] = -SCALE
    ws[0:64, 1, 128] = 2.0 * SCALE
    ws[64:128, 1, 129] = 2.0 * SCALE
    # wk: column r = 2i+p gets -scale on pair-p partitions for DVE slots.
    wk = np.zeros((128, 128), np.float32)
    for i in range(64):
        if not slot_is_act(i):
            wk[0:64, 2 * i] = -SCALE
            wk[64:128, 2 * i + 1] = -SCALE
    bf16 = ml_dtypes.bfloat16
    return ws.astype(bf16), wk.astype(bf16)


def _host_qsum(qc):
    """qc: [128, T] per-core stacked q^T. Returns qs [128, NGROUPS] fp32:
    row r = 2i+p of group g gets -scale*sum_d q[pair p, d, s] for DVE
    slots (s = 64g+i), 0 for ACT slots."""
    import ml_dtypes

    qb = qc.astype(ml_dtypes.bfloat16).astype(np.float64)
    qsum = qb.reshape(2, 64, T).sum(axis=1)  # [pair, s]
    qs = np.zeros((128, NGROUPS), np.float64)
    for g in range(NGROUPS):
        for i in range(64):
            if not slot_is_act(i):
                s = 64 * g + i
                for p in range(2):
                    qs[2 * i + p, g] = -SCALE * qsum[p, s]
    return qs.astype(np.float32)


def get_module(reps=1):
    key = ("nc", reps)
    nc = _cached.get(key)
    if nc is None:
        nc = _build_module(reps)
        _cached[key] = nc
    return nc


def make_in_maps(q, k):
    """Shard full [B,T,H,D] q/k into 8 per-core input maps."""
    q = np.asarray(q, dtype=np.float32)
    k = np.asarray(k, dtype=np.float32)
    # [B, T, H, D] -> [B, H, D, T] -> [B*H, D, T]
    qt = np.ascontiguousarray(q.transpose(0, 2, 3, 1)).reshape(B * H, D, T)
    kt = np.ascontiguousarray(k.transpose(0, 2, 3, 1)).reshape(B * H, D, T)
    ws, wk = _host_weights()
    in_maps = []
    import ml_dtypes

    for c in range(NCORES):
        qc = np.ascontiguousarray(qt[2 * c : 2 * c + 2].reshape(128, T))
        kc = np.ascontiguousarray(
            kt[2 * c : 2 * c + 2].reshape(128, T)
        ).astype(ml_dtypes.bfloat16)
        in_maps.append(
            {"q": qc, "k": kc, "ws": ws, "wk": wk, "qs": _host_qsum(qc)}
        )
    return in_maps


def assemble_output(core_outs):
    """core_outs: list of 8 arrays [NGROUPS, 128, T] -> full [B, T, T, H]."""
    outf = np.empty((B, T, T, H), np.float32)
    for c in range(NCORES):
        o = np.asarray(core_outs[c]).reshape(NGROUPS, 64, 2, T)
        # row r = 2i+p in group g  ->  query s = 64g + i, pair p
        o = o.transpose(2, 0, 1, 3).reshape(2, T, T)
        for p in range(2):
            pg = 2 * c + p
            b, h = divmod(pg, H)
            outf[b, :, :, h] = o[p]
    return outf


def kernel(q, k):
    from concourse.bass_utils import run_bass_kernel_spmd

    nc = get_module()
    in_maps = make_in_maps(q, k)
    res = run_bass_kernel_spmd(
        nc, in_maps, core_ids=list(range(NCORES)), trace=False
    )
    _cached["last_results"] = res
    return assemble_output([r["out"] for r in res.results])
